# revision 50
# baseline (speedup 1.0000x reference)
"""Trainium2 Bass kernel for nn_Attention_49606872268904.

Dense causal GQA attention block (B=1, S=2048, D=4096, 32 q-heads, 8 kv-heads,
head_dim=128, rope, causal mask, output projection), tensor-parallel over heads
across 8 NeuronCores: core c owns q-heads 4c..4c+3 and kv-head c.

v2 design (bf16 everywhere, AllGather epilogue):
- All matmul operands are bf16 (host-precast); PSUM accumulation stays fp32.
  Error budget: measured ~4e-3 scale-relative vs the 2e-2 gate.
- Phase P: per seq group, x tile [128, 32x512] streamed once; 6 output tiles
  (4 q heads + k + v) accumulate over 32 contraction chunks. KV matmuls are
  emitted before Q matmuls each group so the PE stays busy while the previous
  group's rope evacuations run on DVE.
- RoPE via permuted-weight trick: wq/wk rows reordered per head to
  [real_0..63, imag_0..63]; rotation is two contiguous half-tile muls.
- Phase A: per query group (descending size order), per head: score matmul ->
  exp (scalar, bf16 out) -> AV + denominator matmuls. Blocks strictly above
  the causal diagonal are skipped; diagonal blocks masked multiplicatively.
- Epilogue: per query group the 4 heads' normalized attn outputs ([512, 512]
  bf16 = 0.5 MB) are AllGathered across the 8 cores; each core then computes
  its 512 output rows of wo against the full gathered activations. This
  replaces the old 32 MB fp32 ReduceScatter (16x less collective traffic).
- Output returned as bf16 and upcast on host (lossless for values already
  rounded through bf16).
"""

import numpy as np
import ml_dtypes

import concourse.bass as bass
import concourse.mybir as mybir
import concourse.tile as tile
from concourse import bacc
from concourse.bass_utils import run_bass_kernel_spmd
from concourse.masks import make_identity, make_upper_triangular

B, S, DIM = 1, 2048, 4096
NH, NKV, HD = 32, 8, 128
N_CORES = 8
HPC = NH // N_CORES          # 4 q heads per core
OPC = HPC * HD               # 512 output dims per core
DCH = DIM // 128             # 32 contraction chunks
SW = 512                     # seq group width
NSG = S // SW                # 4 seq groups
SCALE = float(HD) ** -0.5

DT = mybir.dt.float32
BF = mybir.dt.bfloat16
F8 = mybir.dt.float8e4
DR = mybir.MatmulPerfMode.DoubleRow
FP = mybir.ActivationFunctionType

_cached = None
last_results = None  # BassKernelResults of the most recent run (for test harness)


def build_program():
    nc = bacc.Bacc(
        "TRN2",
        target_bir_lowering=False,
        debug=False,
        enable_asserts=False,
        num_devices=N_CORES,
    )

    xP = nc.declare_dram_parameter("xP", [128, NSG, DCH, SW], BF, isOutput=False)
    x8 = nc.declare_dram_parameter("x8", [128, NSG, DCH // 2, 2, SW], F8, isOutput=False)
    w8k = nc.declare_dram_parameter("w8k", [128, DCH // 2, 2, HD], F8, isOutput=False)
    w8q = nc.declare_dram_parameter("w8q", [128, DCH // 2, 2, OPC], F8, isOutput=False)
    wvP = nc.declare_dram_parameter("wvP", [128, DCH, HD], BF, isOutput=False)
    woP = nc.declare_dram_parameter("woP", [128, DCH, OPC], BF, isOutput=False)
    cos2 = nc.declare_dram_parameter("cos2", [128, S], DT, isOutput=False)
    sinpm = nc.declare_dram_parameter("sinpm", [128, S], DT, isOutput=False)
    y_out = nc.declare_dram_parameter("y_shard", [4, 128, NSG, SW], BF, isOutput=True)

    with tile.TileContext(nc) as tc:
        with (
            tc.tile_pool(name="dram", bufs=1, space="DRAM") as dram,
            tc.tile_pool(name="consts", bufs=1) as consts,
            tc.tile_pool(name="persist", bufs=1) as persist,
        ):
            attn_sh = [dram.tile([OPC, SW], BF, name=f"ash{qt}") for qt in range(NSG)]
            ag_out = [
                dram.tile([NH * HD, SW], BF, name=f"ago{qt}", addr_space="Shared")
                for qt in range(NSG)
            ]
            # t=0 barrier: absorbs the runtime's staggered core starts during
            # phase P so the first real AllGather's rendezvous is cheap
            bar_in = dram.tile([8, 16], BF, name="bar_in")
            bar_out = dram.tile([64, 16], BF, name="bar_out", addr_space="Shared")
            nc.gpsimd.collective_compute(
                "AllGather",
                mybir.AluOpType.bypass,
                replica_groups=[list(range(N_CORES))],
                ins=[bar_in[:]],
                outs=[bar_out[:]],
            )

            ident = consts.tile([128, 128], BF)
            make_identity(nc, ident)
            tri_keep = consts.tile([128, 128], BF)
            make_upper_triangular(nc, tri_keep, val=1.0, diag=True)
            ones_f = consts.tile([128, 128], DT)
            nc.gpsimd.memset(ones_f, 1.0)
            ones_mat = consts.tile([128, 128], BF)
            nc.vector.tensor_copy(ones_mat, ones_f)
            # scalar HWDGE ring: keeps rope tables + V weights off the sync
            # ring that gates the first fp8 matmuls
            cos2_sb = consts.tile([128, S], DT)
            nc.scalar.dma_start(cos2_sb, cos2[:])
            sinpm_sb = consts.tile([128, S], DT)
            nc.scalar.dma_start(sinpm_sb, sinpm[:])

            KT = persist.tile([128, S], BF)       # K_rot^T, all kv positions
            V = persist.tile([128, S], BF)        # V block [kv, hd] at col 128j
            q_tiles = {}                          # (sg, h) -> [128, SW] bf16

            # ---------------- Phase P: QKV projections + RoPE ----------------
            with (
                nc.named_scope("phaseP"),
                tc.tile_pool(name="psP", bufs=1, space="PSUM") as psP,
                tc.tile_pool(name="sbP", bufs=1) as sbP,
            ):
                # fp8 K weights first (smallest, gate the very first matmuls),
                # then fp8 Q weights; bf16 V weights ride the scalar ring.
                w8k_sb = sbP.tile([128, (DCH // 2) * 2 * HD], F8)         # 4KB
                nc.sync.dma_start(
                    w8k_sb.rearrange("p (d two o) -> p d two o", two=2, o=HD),
                    w8k[:],
                )
                w8k_v = w8k_sb.rearrange("p (d two o) -> p d two o", two=2, o=HD)
                w8q_sb = sbP.tile([128, (DCH // 2) * 2 * OPC], F8)        # 16KB
                w8q_v = w8q_sb.rearrange("p (d two o) -> p d two o", two=2, o=OPC)
                wv_sb = sbP.tile([128, DCH * HD], BF)      # 8KB/part
                nc.scalar.dma_start(
                    wv_sb.rearrange("p (d o) -> p d o", o=HD), wvP[:]
                )
                wv_v = wv_sb.rearrange("p (d o) -> p d o", o=HD)

                for sg in range(NSG):
                    scol = slice(sg * SW, (sg + 1) * SW)
                    x8g = sbP.tile([128, DCH * SW], F8, tag="x8g", bufs=2, name=f"x8g{sg}")
                    x8_v = x8g.rearrange("p (d two s) -> p d two s", two=2, s=SW)
                    if sg == 0:
                        # finest split first: the very first K matmul only
                        # waits on a 0.25MB piece
                        for pc in range(4):
                            nc.sync.dma_start(
                                x8_v[:, 4 * pc : 4 * pc + 4], x8[:, sg, 4 * pc : 4 * pc + 4]
                            )
                    else:
                        nc.sync.dma_start(x8_v[:, 0:8], x8[:, sg, 0:8])
                        nc.sync.dma_start(x8_v[:, 8:16], x8[:, sg, 8:16])
                    if sg == 0:
                        # Q weights after the first fp8 x piece: K matmuls can
                        # begin while these stream in
                        nc.sync.dma_start(
                            w8q_sb.rearrange("p (d two o) -> p d two o", two=2, o=OPC),
                            w8q[:],
                        )
                    xg = sbP.tile([128, DCH * SW], BF, tag="xg", bufs=2, name=f"xg{sg}")
                    xg_v = xg.rearrange("p (d s) -> p d s", s=SW)
                    nc.sync.dma_start(xg_v[:, 0:16], xP[:, sg, 0:16])
                    nc.sync.dma_start(xg_v[:, 16:32], xP[:, sg, 16:32])
                    k_ps = psP.tile([128, SW], DT, tag="k", bufs=2, name=f"kps{sg}")
                    v_ps = psP.tile([128, SW], DT, tag="v", bufs=1, name=f"vps{sg}")
                    q_ps = [
                        psP.tile([128, SW], DT, tag=f"q{h}", bufs=1, name=f"qps{sg}{h}")
                        for h in range(HPC)
                    ]
                    # sg0: fp8 K/Q first (small DMAs gate them); later groups:
                    # bf16 V first so PE has work while the previous group's
                    # rope evacuations drain on DVE.
                    def v_loop():
                        for d in range(DCH):
                            nc.tensor.matmul(
                                v_ps, wv_v[:, d], xg[:, d * SW : (d + 1) * SW],
                                start=(d == 0), stop=(d == DCH - 1),
                            )

                    def kq_loop():
                        for d2 in range(DCH // 2):
                            nc.tensor.matmul(
                                k_ps, w8k_v[:, d2], x8_v[:, d2],
                                start=(d2 == 0), stop=(d2 == DCH // 2 - 1),
                                perf_mode=DR,
                            )
                        for d2 in range(DCH // 2):
                            for h in range(HPC):
                                nc.tensor.matmul(
                                    q_ps[h], w8q_v[:, d2, :, h * HD : (h + 1) * HD],
                                    x8_v[:, d2],
                                    start=(d2 == 0), stop=(d2 == DCH // 2 - 1),
                                    perf_mode=DR,
                                )

                    if sg == 0:
                        kq_loop()
                        v_loop()
                    else:
                        v_loop()
                        kq_loop()

                    # Evacuations. Scalar does all PSUM->bf16 pre-copies (Copy
                    # table only in this phase); DVE does V-block copies and
                    # rope muls at bf16 2x rate.
                    vtmp = sbP.tile([128, SW], BF, tag="vtmp", bufs=2, name=f"vt{sg}")
                    nc.scalar.copy(vtmp, v_ps)
                    for jj in range(4):
                        j = 4 * sg + jj
                        tr_ps = psP.tile([128, 128], BF, tag="tr", bufs=1, name=f"tr{j}")
                        nc.tensor.transpose(tr_ps, vtmp[:, jj * 128 : (jj + 1) * 128], ident)
                        nc.vector.tensor_copy(V[:, j * 128 : (j + 1) * 128], tr_ps)

                    def rope(ps, out_sb, tag_sfx):
                        # cross-partition reads are only legal from PSUM, so
                        # the rotation reads the fp32 PSUM tile directly
                        t1 = sbP.tile([128, SW], DT, tag="rt1", bufs=2, name=f"t1{tag_sfx}")
                        t2 = sbP.tile([128, SW], DT, tag="rt2", bufs=2, name=f"t2{tag_sfx}")
                        nc.vector.tensor_mul(t1, ps, cos2_sb[:, scol])
                        nc.vector.tensor_mul(t2[0:64], ps[64:128], sinpm_sb[0:64, scol])
                        nc.vector.tensor_mul(t2[64:128], ps[0:64], sinpm_sb[64:128, scol])
                        nc.vector.tensor_add(out_sb, t1, t2)

                    rope(k_ps, KT[:, scol], f"k{sg}")
                    for h in range(HPC):
                        qsb = persist.tile([128, SW], BF, name=f"qsb{sg}{h}")
                        q_tiles[(sg, h)] = qsb
                        rope(q_ps[h], qsb, f"q{sg}{h}")

            # ------- Phases A+W: attention (query groups, big first), then
            # AllGather of attn outputs and the local wo row-slice matmul.
            with (
                tc.tile_pool(name="psA", bufs=1, space="PSUM") as psA,
                tc.tile_pool(name="sbA", bufs=1) as sbA,
                tc.tile_pool(name="psW", bufs=1, space="PSUM") as psW,
                tc.tile_pool(name="sbW", bufs=1) as sbW,
            ):
                wo_sb = sbW.tile([128, DCH * OPC], BF)    # 32KB/part
                nc.sync.dma_start(
                    wo_sb.rearrange("p (d o) -> p d o", o=OPC), woP[:]
                )
                wo_v = wo_sb.rearrange("p (d o) -> p d o", o=OPC)

                def phase_a(qt):
                    nb = 4 * qt + 4
                    with nc.named_scope(f"phaseA{qt}"):
                        for h in range(HPC):
                            attn_ps = psA.tile([128, SW], DT, tag="attn", bufs=3, name=f"aps{qt}{h}")
                            # lhsT = [128,128] ones -> den replicated on all
                            # 128 partitions: full-width reciprocal, no
                            # partition_broadcast needed
                            den_ps = psA.tile([128, SW], DT, tag="den", bufs=1, name=f"dps{qt}{h}")
                            for j in range(nb):
                                kk = j - 4 * qt
                                off = 128 * kk if kk > 0 else 0
                                s_ps = psA.tile([128, SW], DT, tag="s", bufs=2, name=f"sps{qt}{h}{j}")
                                nc.tensor.matmul(
                                    s_ps[:, off:],
                                    KT[:, j * 128 : (j + 1) * 128],
                                    q_tiles[(qt, h)][:, off:],
                                    start=True, stop=True,
                                )
                                exp_sb = sbA.tile([128, SW], BF, tag="exp", bufs=4, name=f"ex{qt}{h}{j}")
                                nc.scalar.activation(
                                    exp_sb[:, off:], s_ps[:, off:], FP.Exp, scale=SCALE
                                )
                                if kk >= 0:  # diagonal block: zero kv > q triangle
                                    nc.vector.tensor_mul(
                                        exp_sb[:, off : off + 128],
                                        exp_sb[:, off : off + 128],
                                        tri_keep,
                                    )
                                nc.tensor.matmul(
                                    attn_ps[:, off:],
                                    V[:, j * 128 : (j + 1) * 128],
                                    exp_sb[:, off:],
                                    start=(j == 0), stop=(j == nb - 1),
                                )
                                nc.tensor.matmul(
                                    den_ps[:, off:],
                                    ones_mat,
                                    exp_sb[:, off:],
                                    start=(j == 0), stop=(j == nb - 1),
                                )
                            rd_bc = sbA.tile([128, SW], DT, tag="rdbc", bufs=3, name=f"rdb{qt}{h}")
                            # ~0.7us vs 3.4us for exact reciprocal; den is
                            # a sum of ~1e3 positive O(1) terms, no edge cases
                            nc.vector.reciprocal_approx_fast(rd_bc, den_ps)
                            attn_bf = sbA.tile([128, SW], BF, tag="abf", bufs=3, name=f"abf{qt}{h}")
                            nc.vector.tensor_mul(attn_bf, attn_ps, rd_bc)
                            nc.sync.dma_start(
                                attn_sh[qt][h * 128 : (h + 1) * 128, :], attn_bf
                            )
                        nc.gpsimd.collective_compute(
                            "AllGather",
                            mybir.AluOpType.bypass,
                            replica_groups=[list(range(N_CORES))],
                            ins=[attn_sh[qt][:]],
                            outs=[ag_out[qt][:]],
                        )

                def phase_w(qt):
                    with nc.named_scope(f"phaseW{qt}"):
                        agq = sbW.tile([128, DCH * SW], BF, tag="agq", bufs=2, name=f"agq{qt}")
                        agr = ag_out[qt].rearrange("(d p) s -> p d s", p=128)
                        agv = agq.rearrange("p (d s) -> p d s", s=SW)
                        # 4 piece reads on the scalar HWDGE ring: the first W
                        # matmul only waits on the first 1MB piece
                        for pc in range(4):
                            nc.scalar.dma_start(
                                agv[:, pc * 8 : (pc + 1) * 8], agr[:, pc * 8 : (pc + 1) * 8]
                            )
                        for t in range(4):
                            yp = psW.tile([128, SW], DT, tag="yp", bufs=2, name=f"yp{qt}{t}")
                            for d in range(DCH):
                                nc.tensor.matmul(
                                    yp,
                                    wo_v[:, d, t * 128 : (t + 1) * 128],
                                    agq[:, d * SW : (d + 1) * SW],
                                    start=(d == 0), stop=(d == DCH - 1),
                                )
                            y_sb = sbW.tile([128, SW], BF, tag="ysb", bufs=3, name=f"ysb{qt}{t}")
                            nc.vector.tensor_copy(y_sb, yp)
                            nc.sync.dma_start(y_out[t][:, qt], y_sb)

                # ascending: AG(0) fires earliest, so the serialized
                # collective chain (gpsimd blocks on each completion) starts
                # as soon as possible; each W(qt) then has >=25us of slack
                phase_a(0)
                phase_a(1)
                phase_a(2)
                phase_a(3)
                phase_w(0)
                phase_w(1)
                phase_w(2)
                phase_w(3)

    nc.compile()
    return nc


def _get_program():
    global _cached
    if _cached is None:
        _cached = build_program()
    return _cached


_ROPE_PERM = np.concatenate([np.arange(0, HD, 2), np.arange(1, HD, 2)])
_BFNP = ml_dtypes.bfloat16
_F8NP = ml_dtypes.float8_e4m3fn


def kernel(**inputs):
    x = np.asarray(inputs["x"], np.float32)
    wq = np.asarray(inputs["wq"], np.float32)
    wk = np.asarray(inputs["wk"], np.float32)
    wv = np.asarray(inputs["wv"], np.float32)
    wo = np.asarray(inputs["wo"], np.float32)
    fc = np.asarray(inputs["freqs_cos"], np.float32)
    fs = np.asarray(inputs["freqs_sin"], np.float32)

    cosT = np.ascontiguousarray(fc.T)                        # [64, S]
    sinT = np.ascontiguousarray(fs.T)
    cos2 = np.concatenate([cosT, cosT], axis=0)              # [128, S]
    sinpm = np.concatenate([-sinT, sinT], axis=0)
    # x blocked [128 p, sg, d, s]
    xP = np.ascontiguousarray(
        x.reshape(NSG, SW, DCH, 128).transpose(3, 0, 2, 1)
    ).astype(_BFNP)
    x8_h = xP.astype(_F8NP).reshape(128, NSG, DCH // 2, 2, SW)

    in_maps = []
    for c in range(N_CORES):
        wq_c = wq[c * OPC : (c + 1) * OPC].reshape(HPC, HD, DIM)[:, _ROPE_PERM].reshape(OPC, DIM)
        wk_c = wk[c * HD : (c + 1) * HD][_ROPE_PERM]
        wv_c = wv[c * HD : (c + 1) * HD]
        wstack = np.concatenate([wq_c, wk_c, wv_c], axis=0)  # [768, DIM]
        wqkv_c = np.ascontiguousarray(
            wstack.T.reshape(DCH, 128, 768).transpose(1, 0, 2)
        ).astype(_BFNP)                                      # [128, 32, 768]
        w8_full = wqkv_c[:, :, : OPC + HD].astype(_F8NP)
        w8q_c = np.ascontiguousarray(w8_full[:, :, :OPC]).reshape(128, DCH // 2, 2, OPC)
        w8k_c = np.ascontiguousarray(w8_full[:, :, OPC:]).reshape(128, DCH // 2, 2, HD)
        wv_bf = np.ascontiguousarray(wqkv_c[:, :, OPC + HD :])  # [128, 32, 128]
        wo_c = wo[c * OPC : (c + 1) * OPC, :]                # [512, DIM]
        woP_c = np.ascontiguousarray(
            wo_c.T.reshape(DCH, 128, OPC).transpose(1, 0, 2)
        ).astype(_BFNP)                                      # [128, 32, 512]
        in_maps.append(
            dict(xP=xP, x8=x8_h, w8k=w8k_c, w8q=w8q_c, wvP=wv_bf, woP=woP_c,
                 cos2=cos2, sinpm=sinpm)
        )

    nc = _get_program()
    res = run_bass_kernel_spmd(nc, in_maps, list(range(N_CORES)))
    global last_results
    last_results = res

    yT = np.empty((DIM, S), np.float32)
    for c in range(N_CORES):
        shard = res.results[c]["y_shard"]                    # [4, 128, 4, 512] bf16
        yT[c * OPC : (c + 1) * OPC] = np.asarray(shard, _BFNP).astype(np.float32).reshape(OPC, S)
    return np.ascontiguousarray(yT.T).reshape(B, S, DIM)
